# revision 62
# baseline (speedup 1.0000x reference)
"""Segment-normalize kernel for trn2, 8 NeuronCores, batch-parallel.

v2 design (cost-model driven):
- fp32 input, bf16 output (output error is relative; input must stay fp32
  because the stats need ~2e-5 absolute accuracy near zero outputs).
- Per core: x4 mega-tile [128, 4, 8192] (4 units = 2 batches x 2 feature
  halves), loaded in 16 S-slices of 512 so stats start early.
- VE: one bn_stats per (segment, unit) -- the only one-pass sum+sumsq op.
- Pool (gpsimd): stats->mean/var combine + A/C scalar assembly, in NB
  geometric batches along S, plus a share of the normalize instructions.
- ACT: Rsqrt batches + the bulk of the normalize (per-segment scale/bias
  activation, bf16 out).
- Stores: bf16 y in 8 y-slices of 1024 from a 3-deep buffer pool.
- Segments shorter than T_HOST are normalized on the host (they are pure
  per-instruction overhead on device: ~2.7% of positions); device skips
  their stats/norm instructions, host overwrites those y positions.

The device program is specialized at trace time on the segment boundary
list; compiled NEFFs are cached per boundary signature.
"""

import numpy as np
import ml_dtypes

import concourse.bass as bass
from concourse import mybir
from concourse.bass_utils import run_bass_kernel_spmd

B, S, F = 16, 8192, 256
NCORES = 8
BPC = B // NCORES            # batches per core
NUNITS = BPC * 2             # (batch, feature-half) units per core
EPS = 1e-5
PIECE = 512                  # bn_stats hardware max free size
LSLICE = 512                 # load slice length
NLS = S // LSLICE            # 16 load slices
YSLICE = 1024                # store slice length
NYS = S // YSLICE            # 8 y slices
YBUFS = 3                    # y buffer pool depth (>=3 to avoid deadlock)
T_HOST = 32                  # segments shorter than this are done on host
BATCH_SLICES = [(0, 2), (2, 4), (4, 7), (7, 10), (10, 13), (13, 16)]
VE_NORM_LAST = 6             # VE eligible for norm tasks in the last N batches
ACT_SKIP_LAST = 2            # ACT excluded from norm tasks in the last N batches

# measured TimelineSim per-instruction costs (ns), for the greedy balancer
def _c_act_norm(L):
    return 0.833 * L + 195.0

def _c_pool_norm(L):
    return 1.389 * L + 110.0

def _c_ve_norm(L):
    return 0.52 * L + 70.0

def _c_ve_stats(L):
    n = -(-L // PIECE)
    return 1.0417 * L + 70.0 * n

_cache: dict = {}


class Plan:
    pass


def _plan(change_points: np.ndarray, trivial_affine: bool = True) -> Plan:
    cp = np.asarray(change_points)
    ind = (cp.sum(axis=0) > 0)
    ind[0] = False
    bpos = np.flatnonzero(ind)
    starts = np.concatenate([[0], bpos]).astype(np.int64)
    ends = np.concatenate([bpos, [S]]).astype(np.int64)
    segs = [(int(s), int(e - s)) for s, e in zip(starts, ends)]

    p = Plan()
    p.trivial_affine = trivial_affine
    p.segs = segs
    p.short = [k for k, (s0, ln) in enumerate(segs) if ln < T_HOST]
    dev = [k for k, (s0, ln) in enumerate(segs) if ln >= T_HOST]

    # pieces: (s0, plen, seg_k, is_main); main piece = last piece of the seg
    pieces = []
    for k in dev:
        s0, ln = segs[k]
        off = 0
        while off < ln:
            pl = min(PIECE, ln - off)
            pieces.append([s0 + off, pl, k, off + pl == ln])
            off += pl

    # batches over load slices; merge so every batch has >= 1 main piece
    def end_slice(s0, pl):
        return (s0 + pl - 1) // LSLICE

    raw_batches = list(BATCH_SLICES)
    batches = []
    pend = []
    for (a, b) in raw_batches:
        pend.append((a, b))
        n_main = sum(1 for (s0, pl, k, m) in pieces
                     if m and pend[0][0] <= end_slice(s0, pl) < pend[-1][1])
        if n_main > 0:
            batches.append((pend[0][0], pend[-1][1]))
            pend = []
    if pend:  # leftover empty range: merge into last batch
        if batches:
            batches[-1] = (batches[-1][0], pend[-1][1])
        else:
            batches.append((pend[0][0], pend[-1][1]))
    p.batches = batches
    NB = len(batches)

    def batch_of_slice(sl):
        for bi, (a, b) in enumerate(batches):
            if a <= sl < b:
                return bi
        return NB - 1

    # order cols by (batch, position)
    pieces.sort(key=lambda t: (batch_of_slice(end_slice(t[0], t[1])), t[0]))
    p.cols = pieces                       # col j = pieces[j]
    ncol = len(pieces)
    p.ncol = ncol
    p.ncolp = (ncol + 3) // 4 * 4
    p.col_of_seg = {}
    for j, (s0, pl, k, m) in enumerate(pieces):
        if m:
            p.col_of_seg[k] = j
    p.extras = []                          # (main_col, extra_col)
    for j, (s0, pl, k, m) in enumerate(pieces):
        if not m:
            p.extras.append((p.col_of_seg[k], j))

    # batch -> col range (cols are batch-contiguous by construction)
    p.bat_cols = []
    for bi in range(NB):
        js = [j for j, (s0, pl, k, m) in enumerate(pieces)
              if batch_of_slice(end_slice(s0, pl)) == bi]
        p.bat_cols.append((min(js), max(js) + 1) if js else (0, 0))
    # make ranges non-overlapping ascending; extras of later batches may sit
    # inside earlier ranges only if seg spans batches -- main piece defines
    # the batch where fold + mean/var happen, but its extras' wide values
    # must be computed by then. Assign every piece to the batch of its own
    # end slice (done above); a main piece is always in the latest batch of
    # its seg, so extras are computed in the same or earlier batch. Folds
    # happen in the main's batch.
    p.fold_in_batch = [[] for _ in range(NB)]
    for (mj, ej) in p.extras:
        s0, pl, k, m = pieces[mj]
        p.fold_in_batch[batch_of_slice(end_slice(s0, pl))].append((mj, ej))

    # VE stats groups per load slice: (slice, [(col, s0, pl)]), inc VST
    p.slice_groups = []
    for sl in range(NLS):
        g = [(j, s0, pl) for j, (s0, pl, k, m) in enumerate(pieces)
             if end_slice(s0, pl) == sl]
        g.sort(key=lambda t: t[1])
        p.slice_groups.append(g)
    p.vst_cum = np.cumsum([1 if g else 0 for g in p.slice_groups]).tolist()

    # const arrays per col (x4 unit slots): cea = ce/Lseg, coa = co/Lseg,
    # invl = 1/Lseg
    ncolp = p.ncolp
    cea = np.zeros((ncolp, 4), np.float32)
    coa = np.zeros((ncolp, 4), np.float32)
    invl = np.zeros((ncolp, 4), np.float32)
    for j, (s0, pl, k, m) in enumerate(pieces):
        Lseg = segs[k][1]
        cea[j, :] = ((pl + 1) // 2) / Lseg
        coa[j, :] = (pl // 2) / Lseg
        invl[j, :] = 1.0 / Lseg
    p.cea = cea.reshape(-1)
    p.coa = coa.reshape(-1)
    p.invl = invl.reshape(-1)

    # normalize tasks: per (dev seg, unit), split at YSLICE boundaries
    # task = (batch, ys, u, gs, L, col)
    tasks = []
    for k in dev:
        s0, ln = segs[k]
        col = p.col_of_seg[k]
        bi = batch_of_slice(end_slice(s0 + ln - min(ln, PIECE),
                                      min(ln, PIECE)) if False else
                            (s0 + ln - 1) // LSLICE)
        a = s0
        while a < s0 + ln:
            e = min(s0 + ln, (a // YSLICE + 1) * YSLICE)
            for u in range(4):
                tasks.append((bi, a // YSLICE, u, a, e - a, col))
            a = e

    # time-aware engine assignment (ACT / Pool / VE-for-late-batches):
    # model per-engine clock, batch-ready times (VE stats pace + combine
    # chain), and VE's availability only after its stats stream ends.
    load_t = []
    tl = 2.33
    for s in range(NLS):
        if s == 2:
            tl += 2.0
        tl += 2.913
        load_t.append(tl)

    nw = 16 if p.trivial_affine else 23   # combine+recip+AC VE instrs

    def combine_cost_ve(bi):
        c0, c1 = p.bat_cols[bi]
        E = (c1 - c0) * 4
        return (nw * 60 + (nw + 1) * E * 1.0417
                + len(p.fold_in_batch[bi]) * 2 * 70) / 1000.0

    # VE pace: stats per slice + combine at batch boundaries
    bnd = {p.batches[b][1] - 1: b for b in range(NB)}
    stats_done = []
    ready = [0.0] * NB
    tve = 5.2
    for s in range(NLS):
        if p.slice_groups[s]:
            tve = max(tve, load_t[s] + 0.9)
            tve += sum(_c_ve_stats(pl) * 4
                       for (j, s0, pl) in p.slice_groups[s]) / 1000.0
        if s in bnd:
            tve += combine_cost_ve(bnd[s])
            ready[bnd[s]] = tve
        stats_done.append(tve)
    stats_end = tve
    free = {'act': 3.0, 'pool': 3.0, 've': stats_end}
    act_tasks, pool_tasks, ve_tasks = [], [], []
    out = {'act': act_tasks, 'pool': pool_tasks, 've': ve_tasks}
    cost = {'act': _c_act_norm, 'pool': _c_pool_norm, 've': _c_ve_norm}
    for bi in range(NB):
        # A/C of batch bi is emitted after combine(bi+1) on VE
        unlock = ready[min(bi + 1, NB - 1)] + 0.5
        for e in ('act', 'pool', 've'):
            free[e] = max(free[e], unlock)
        bt = sorted([t for t in tasks if t[0] == bi], key=lambda t: -t[4])
        for t in bt:
            cand = ['act', 'pool']
            if bi >= NB - VE_NORM_LAST:
                cand.append('ve')
            e = min(cand, key=lambda e: free[e] + cost[e](t[4]) / 1000.0)
            free[e] += cost[e](t[4]) / 1000.0
            out[e].append(t)
    key = lambda t: (t[0], t[1], t[3])
    act_tasks.sort(key=key)
    pool_tasks.sort(key=key)
    ve_tasks.sort(key=key)
    p.act_tasks = act_tasks
    p.pool_tasks = pool_tasks
    p.ve_tasks = ve_tasks

    # per-engine cumulative NY counts for each y slice: position (1-based) of
    # the last instr with ys <= target in the engine stream
    def cum_counts(tl):
        c = [0] * NYS
        for i, t in enumerate(tl):
            for ys in range(t[1], NYS):
                c[ys] = i + 1
        return c
    p.ca = cum_counts(act_tasks)
    p.cp = cum_counts(pool_tasks)
    p.cv = cum_counts(ve_tasks)

    p.sig = tuple(s for s, _ in segs) + (T_HOST, trivial_affine)
    return p


def _build(p: Plan):
    f32 = mybir.dt.float32
    bf16 = mybir.dt.bfloat16
    nc = bass.Bass()
    ncolp = p.ncolp
    NB = len(p.batches)

    import os
    dbg = bool(os.environ.get("KRN_DEBUG_DUMP"))
    xt4 = nc.declare_dram_parameter("xt4", [4, 128, S], f32, isOutput=False)
    if dbg:
        dbg_s6 = nc.declare_dram_parameter("dbg_s6", [128, p.ncolp, 4, 6], f32, isOutput=True)
        dbg_rstd = nc.declare_dram_parameter("dbg_rstd", [128, p.ncolp, 4], f32, isOutput=True)
        dbg_c4 = nc.declare_dram_parameter("dbg_c4", [128, p.ncolp, 4], f32, isOutput=True)
    cea_d = nc.declare_dram_parameter("cea", [ncolp * 4], f32, isOutput=False)
    coa_d = nc.declare_dram_parameter("coa", [ncolp * 4], f32, isOutput=False)
    invl_d = nc.declare_dram_parameter("invl", [ncolp * 4], f32, isOutput=False)
    wb_d = nc.declare_dram_parameter("wb", [128, 8], f32, isOutput=False)
    yt4 = nc.declare_dram_parameter("yt4", [4, 128, S], bf16, isOutput=True)

    from contextlib import ExitStack
    ctx = ExitStack()
    with ctx:
        x4 = ctx.enter_context(nc.sbuf_tensor([128, 4, S], f32))
        yb = [ctx.enter_context(nc.sbuf_tensor(f"yb{i}", [128, 4, YSLICE], bf16))
              for i in range(YBUFS)]
        s6 = ctx.enter_context(nc.sbuf_tensor([128, ncolp, 4, 6], f32))
        mean = ctx.enter_context(nc.sbuf_tensor([128, ncolp, 4], f32))
        vv = ctx.enter_context(nc.sbuf_tensor([128, ncolp, 4], f32))
        rstd = ctx.enter_context(nc.sbuf_tensor([128, ncolp, 4], f32))
        A4 = rstd if p.trivial_affine else \
            ctx.enter_context(nc.sbuf_tensor([128, ncolp, 4], f32))
        # C4 aliases mean: C4 is written from t1 after mean's last read
        # (t1 = mean*rstd), batch ranges disjoint
        C4 = mean
        t1 = ctx.enter_context(nc.sbuf_tensor([128, ncolp, 4], f32))
        t2 = ctx.enter_context(nc.sbuf_tensor([128, ncolp, 4], f32))
        # var aliases t2: t2's last read in combine precedes the var write,
        # and batch col-ranges are disjoint so cross-batch pipelining is safe
        var = t2
        cea_t = ctx.enter_context(nc.sbuf_tensor([128, ncolp, 4], f32))
        coa_t = ctx.enter_context(nc.sbuf_tensor([128, ncolp, 4], f32))
        invl_t = ctx.enter_context(nc.sbuf_tensor([128, ncolp, 4], f32))
        wb_t = ctx.enter_context(nc.sbuf_tensor([128, 8], f32))
        # 4 rotating load sems: DMA completions are unordered across
        # in-flight instructions, so a single counting sem cannot express
        # "slice s landed". With rotation + a predecessor wait at issue
        # time, reaching 16*(s//4+1) on LDs[s%4] implies slice s is loaded.
        LDs = [ctx.enter_context(nc.semaphore(f"LD{i}")) for i in range(4)]
        LDC = ctx.enter_context(nc.semaphore("LDC"))
        VST = ctx.enter_context(nc.semaphore("VST"))
        RCP = ctx.enter_context(nc.semaphore("RCP"))
        PCB = ctx.enter_context(nc.semaphore("PCB"))
        RSQ = ctx.enter_context(nc.semaphore("RSQ"))
        ACB = ctx.enter_context(nc.semaphore("ACB"))
        NYA = ctx.enter_context(nc.semaphore("NYA"))
        NYP = ctx.enter_context(nc.semaphore("NYP"))
        NYV = ctx.enter_context(nc.semaphore("NYV"))
        ST = ctx.enter_context(nc.semaphore("ST"))
        block = ctx.enter_context(nc.Block())

        def bcast(dram, n):
            ap = dram[:]
            return bass.AP(tensor=ap.tensor, offset=ap.offset,
                           ap=[[0, 128], [1, n]])

        # y-slices >= YBUFS live in the dead fp32 x region of y-slice
        # (ys - YBUFS): by the time any engine writes y-slice ys, all
        # stats/norm reads of that x region have completed (NY waits).
        xb16 = x4.bitcast(bf16)   # [128, 4, 2*S]

        def y_ap(ys, u, loc, L):
            if ys < YBUFS:
                return yb[ys][:, u, loc:loc + L]
            o = (ys - YBUFS) * 2 * YSLICE + loc
            return xb16[:, u, o:o + L]

        def y_store_ap(ys):
            if ys < YBUFS:
                return yb[ys][:, :, :]
            o = (ys - YBUFS) * 2 * YSLICE
            return xb16[:, :, o:o + YSLICE]

        def guard_waits(eng, ys, skip):
            # x region (ys - YBUFS) must be fully consumed by all engines
            r = ys - YBUFS
            if r < 0:
                return
            if skip != 'act' and p.ca[r]:
                eng.wait_ge(NYA, p.ca[r])
            if skip != 'pool' and p.cp[r]:
                eng.wait_ge(NYP, p.cp[r])
            if skip != 've' and p.cv[r]:
                eng.wait_ge(NYV, p.cv[r])

        @block.sync
        def _(sp):
            for sl in range(NLS):
                if sl == 2:
                    sp.dma_start(out=cea_t[:, :, :],
                                 in_=bcast(cea_d, ncolp * 4)).then_inc(LDC, 16)
                    sp.dma_start(out=coa_t[:, :, :],
                                 in_=bcast(coa_d, ncolp * 4)).then_inc(LDC, 16)
                    sp.dma_start(out=invl_t[:, :, :],
                                 in_=bcast(invl_d, ncolp * 4)).then_inc(LDC, 16)
                    sp.dma_start(out=wb_t[:, :], in_=wb_d[:, :]).then_inc(LDC, 16)
                if sl >= 4:
                    sp.wait_ge(LDs[sl % 4], 16 * (sl // 4))
                sp.dma_start(out=x4[:, :, sl * LSLICE:(sl + 1) * LSLICE],
                             in_=xt4[:, :, sl * LSLICE:(sl + 1) * LSLICE]
                             ).then_inc(LDs[sl % 4], 16)
            for ys in range(NYS):
                if p.ca[ys]:
                    sp.wait_ge(NYA, p.ca[ys])
                if p.cp[ys]:
                    sp.wait_ge(NYP, p.cp[ys])
                if p.cv[ys]:
                    sp.wait_ge(NYV, p.cv[ys])
                sp.dma_start(out=yt4[:, :, ys * YSLICE:(ys + 1) * YSLICE],
                             in_=y_store_ap(ys)).then_inc(ST, 16)
            if dbg:
                nco = p.ncol
                sp.dma_start(out=dbg_s6[:, 0:nco, :, :],
                             in_=s6[:, 0:nco, :, :]).then_inc(ST, 16)
                sp.dma_start(out=dbg_rstd[:, 0:nco, :],
                             in_=rstd[:, 0:nco, :]).then_inc(ST, 16)
                sp.dma_start(out=dbg_c4[:, 0:nco, :],
                             in_=C4[:, 0:nco, :]).then_inc(ST, 16)

        Asc = rstd if p.trivial_affine else A4

        @block.vector
        def _(ve):
            # combine + A/C live on VE: GPSIMD (Pool) has async completion
            # between its own instructions, so RAW-chained tensor math there
            # is racy. VE is an in-order pipeline. combine(b) is emitted at
            # batch b's last slice; recip+A/C of b-1 follow it so the ACT
            # sqrt round trip is off the critical chain by one batch.
            comb_after = {}
            for b in range(NB):
                comb_after.setdefault(p.batches[b][1] - 1, []).append(b)

            def combine(bi):
                c0, c1 = p.bat_cols[bi]
                if bi == 0:
                    ve.wait_ge(LDC, 64)
                # bn_stats writeback is async even on the same engine: a sem
                # round trip (then_inc on the last bn_stats + this wait)
                # forces the s6 writes of this batch to be visible
                ve.wait_ge(VST, bi + 1)
                me = s6[:, c0:c1, :, 1]
                M2e = s6[:, c0:c1, :, 2]
                mo = s6[:, c0:c1, :, 4]
                M2o = s6[:, c0:c1, :, 5]
                ceav = cea_t[:, c0:c1, :]
                coav = coa_t[:, c0:c1, :]
                t1v = t1[:, c0:c1, :]
                t2v = t2[:, c0:c1, :]
                meanv = mean[:, c0:c1, :]
                vvv = vv[:, c0:c1, :]
                v = nc.vector
                v.tensor_mul(out=t1v, in0=me, in1=ceav)
                v.tensor_mul(out=t2v, in0=mo, in1=coav)
                v.tensor_add(out=meanv, in0=t1v, in1=t2v)
                v.tensor_add(out=vvv, in0=M2e, in1=M2o)
                v.tensor_mul(out=vvv, in0=vvv, in1=invl_t[:, c0:c1, :])
                v.tensor_mul(out=t1v, in0=me, in1=me)
                v.tensor_mul(out=t1v, in0=t1v, in1=ceav)
                v.tensor_add(out=vvv, in0=vvv, in1=t1v)
                v.tensor_mul(out=t2v, in0=mo, in1=mo)
                v.tensor_mul(out=t2v, in0=t2v, in1=coav)
                v.tensor_add(out=vvv, in0=vvv, in1=t2v)
                for (mj, ej) in p.fold_in_batch[bi]:
                    v.tensor_add(out=mean[:, mj, :], in0=mean[:, mj, :],
                                 in1=mean[:, ej, :])
                    v.tensor_add(out=vv[:, mj, :], in0=vv[:, mj, :],
                                 in1=vv[:, ej, :])
                v.tensor_mul(out=t1v, in0=meanv, in1=meanv)
                v.tensor_sub(out=var[:, c0:c1, :], in0=vvv,
                             in1=t1v).then_inc(PCB, 1)

            def finish_recip(bi):
                # rstd for batch bi (sqrt done on ACT). The DVE engine keeps
                # several instructions in flight; the slow table-based
                # reciprocal gets overtaken by a back-to-back consumer, so
                # the consumer is gated on RCP (and spaced by combine ops).
                c0, c1 = p.bat_cols[bi]
                ve.wait_ge(RSQ, bi + 1)
                nc.vector.reciprocal(out=rstd[:, c0:c1, :],
                                     in_=vv[:, c0:c1, :]).then_inc(RCP, 1)

            def finish_ac(bi):
                c0, c1 = p.bat_cols[bi]
                t1v = t1[:, c0:c1, :]
                meanv = mean[:, c0:c1, :]
                v = nc.vector
                ve.wait_ge(RCP, bi + 1)
                if p.trivial_affine:
                    # single fused op (no t1 intermediate): C = (mean*-1)*rstd
                    v.scalar_tensor_tensor(
                        out=C4[:, c0:c1, :], in0=meanv, scalar=-1.0,
                        in1=rstd[:, c0:c1, :], op0=mybir.AluOpType.mult,
                        op1=mybir.AluOpType.mult).then_inc(ACB, 1)
                else:
                    for u in range(4):
                        v.tensor_scalar_mul(out=A4[:, c0:c1, u:u + 1],
                                            in0=rstd[:, c0:c1, u:u + 1],
                                            scalar1=wb_t[:, u % 2:u % 2 + 1])
                    v.tensor_mul(out=t1v, in0=meanv, in1=A4[:, c0:c1, :])
                    last = None
                    for u in range(4):
                        last = v.tensor_scalar(out=C4[:, c0:c1, u:u + 1],
                                               in0=t1[:, c0:c1, u:u + 1],
                                               scalar1=-1.0,
                                               scalar2=wb_t[:, 2 + u % 2:3 + u % 2],
                                               op0=mybir.AluOpType.mult,
                                               op1=mybir.AluOpType.add)
                    last.then_inc(ACB, 1)

            last_stats = [None]
            for sl in range(NLS):
                g = p.slice_groups[sl]
                # wait even when the group is empty: a later group's segs may
                # read back into this slice, relying on in-order VE waits
                ve.wait_ge(LDs[sl % 4], 16 * (sl // 4 + 1))
                if g:
                    for (j, s0, pl) in g:
                        for u in range(4):
                            last_stats[0] = nc.vector.bn_stats(
                                out=s6[:, j, u, :], in_=x4[:, u, s0:s0 + pl])
                for b in comb_after.get(sl, []):
                    last_stats[0].then_inc(VST, 1)
                    if b >= 1:
                        finish_recip(b - 1)
                    combine(b)
                    if b >= 1:
                        finish_ac(b - 1)
            finish_recip(NB - 1)
            finish_ac(NB - 1)
            guard = YBUFS - 1
            cur_b = -1
            for (bi, ys, u, gs, L, col) in p.ve_tasks:
                if bi != cur_b:
                    ve.wait_ge(ACB, bi + 1)
                    cur_b = bi
                if ys > guard:
                    guard_waits(ve, ys, 've')
                    guard = ys
                loc = gs - ys * YSLICE
                nc.vector.tensor_scalar(
                    out=y_ap(ys, u, loc, L),
                    in0=x4[:, u, gs:gs + L],
                    scalar1=Asc[:, col, u:u + 1],
                    scalar2=C4[:, col, u:u + 1],
                    op0=mybir.AluOpType.mult,
                    op1=mybir.AluOpType.add).then_inc(NYV, 1)

        @block.scalar
        def _(ac):
            st = {"guard": YBUFS - 1, "ti": 0}

            def sqrt_b(bi):
                c0, c1 = p.bat_cols[bi]
                ac.wait_ge(PCB, bi + 1)
                nc.scalar.activation(
                    out=vv[:, c0:c1, :], in_=var[:, c0:c1, :],
                    func=mybir.ActivationFunctionType.Sqrt,
                    bias=wb_t[:, 4:5], scale=1.0).then_inc(RSQ, 1)

            def norms_b(bi):
                ti = st["ti"]
                if ti < len(p.act_tasks) and p.act_tasks[ti][0] == bi:
                    ac.wait_ge(ACB, bi + 1)
                while ti < len(p.act_tasks) and p.act_tasks[ti][0] == bi:
                    (tb, ys, u, gs, L, col) = p.act_tasks[ti]
                    if ys > st["guard"]:
                        guard_waits(ac, ys, 'act')
                        st["guard"] = ys
                    loc = gs - ys * YSLICE
                    nc.scalar.activation(
                        out=y_ap(ys, u, loc, L),
                        in_=x4[:, u, gs:gs + L],
                        func=mybir.ActivationFunctionType.Identity,
                        scale=Asc[:, col, u:u + 1],
                        bias=C4[:, col, u:u + 1]).then_inc(NYA, 1)
                    ti += 1
                st["ti"] = ti

            for bi in range(NB):
                sqrt_b(bi)
                if bi >= 1:
                    norms_b(bi - 1)
            norms_b(NB - 1)

        @block.gpsimd
        def _(g):
            import os
            if os.environ.get("KRN_DEBUG_MEMSET_Y"):
                for b in yb:
                    g.memset(b[:, :, :], 0.0)
            # Pool runs only independent norm instructions (its cross-
            # instruction RAW ordering is unsafe; these read VE/ACT-written
            # scalars via ACB and write disjoint y ranges)
            guard = YBUFS - 1
            cur_b = -1
            for (bi, ys, u, gs, L, col) in p.pool_tasks:
                if bi != cur_b:
                    g.wait_ge(ACB, bi + 1)
                    cur_b = bi
                if ys > guard:
                    guard_waits(g, ys, 'pool')
                    guard = ys
                loc = gs - ys * YSLICE
                g.tensor_scalar(out=y_ap(ys, u, loc, L),
                                in0=x4[:, u, gs:gs + L],
                                scalar1=Asc[:, col, u:u + 1],
                                scalar2=C4[:, col, u:u + 1],
                                op0=mybir.AluOpType.mult,
                                op1=mybir.AluOpType.add).then_inc(NYP, 1)

    return nc


def _host_short_fix(y, x, w, b, segs, short):
    for k in short:
        s0, ln = segs[k]
        xs = x[:, s0:s0 + ln, :]
        mu = xs.mean(axis=1, keepdims=True)
        v = np.maximum((xs * xs).mean(axis=1, keepdims=True) - mu * mu, 0.0)
        y[:, s0:s0 + ln, :] = ((xs - mu) / np.sqrt(v + EPS)) * w + b


def kernel(x, affine_weight, affine_bias, change_points):
    x = np.asarray(x, dtype=np.float32)
    w = np.asarray(affine_weight, dtype=np.float32)
    bb = np.asarray(affine_bias, dtype=np.float32)
    cp = np.asarray(change_points)

    trivial = bool(np.all(w == 1.0) and np.all(bb == 0.0))
    p = _plan(cp, trivial)
    if p.sig not in _cache:
        _cache[p.sig] = _build(p)
    nc = _cache[p.sig]

    wbarr = np.zeros((128, 8), np.float32)
    wbarr[:, 0] = w[0:128]
    wbarr[:, 1] = w[128:256]
    wbarr[:, 2] = bb[0:128]
    wbarr[:, 3] = bb[128:256]
    wbarr[:, 4] = EPS

    in_maps = []
    for i in range(NCORES):
        xt = np.ascontiguousarray(
            x[i * BPC:(i + 1) * BPC].transpose(0, 2, 1)).reshape(4, 128, S)
        in_maps.append({"xt4": xt, "cea": p.cea, "coa": p.coa,
                        "invl": p.invl, "wb": wbarr})

    res = run_bass_kernel_spmd(nc, in_maps, core_ids=list(range(NCORES)),
                               trace=False)

    y = np.empty((B, S, F), np.float32)
    for i in range(NCORES):
        yt = np.asarray(res.results[i]["yt4"]).astype(np.float32)
        yt = yt.reshape(BPC, 2, 128, S)
        for bi in range(BPC):
            for fh in range(2):
                y[i * BPC + bi, :, fh * 128:(fh + 1) * 128] = yt[bi, fh].T
    if p.short:
        _host_short_fix(y, x, w, bb, p.segs, p.short)
    return y
